# revision 34
# baseline (speedup 1.0000x reference)
"""GNN message passing kernel for Trainium2, v3: raw-Bass engine pipeline.

Same algorithm as v2 (pair-gather + one-hot matmul segment-sum), but
hand-scheduled engine streams with explicit semaphores instead of the
Tile framework:
  - Gathers are PREPARE_ONLY + trigger_dma: GpSimd pays descriptor
    GENERATION only (~5.3ns/idx); the 16 DMA engines drain transfers
    behind the next chunk's generation (completion via s_gdma, +16 per
    chunk).
  - Everything flows in bf16 (embed/agg pair tables, gathered messages,
    W2 one-hot weights) -> half DMA traffic, 1cyc/row PE matmuls, half
    SBUF. PSUM accumulation and out3 stay f32 exact.
  - DVE generates all W2 tiles (one fused tensor_scalar per window,
    bf16 out) into a 16-deep ring; PE waits per-window on s_wd, DVE
    recycles slots on s_mm. No Tile sem quantization.
  - Per hop: ACT copies PSUM->stage(f32), DVE converts stage->bf16,
    SP DMAs out3[h] (f32) + cc_in[h] (bf16), GpSimd runs the bf16
    AllGather into agg[h+1].
"""

import sys

sys.path.insert(0, "/opt/trn_rl_repo")

import numpy as np
import ml_dtypes

import concourse.bacc as bacc
import concourse.mybir as mybir
from concourse.bass_utils import run_bass_kernel_spmd

F32 = mybir.dt.float32
BF16 = mybir.dt.bfloat16
I16 = mybir.dt.int16

N = 50000
E = 800000
D = 64
HOPS = 3
NCORES = 8
NLOC = N // NCORES           # 6250
NTILE = (NLOC + 127) // 128  # 49
NPAD = NTILE * 128           # 6272
CW = 56                      # windows per gather chunk (7168 idx; 448 descs/engine so two chunks fit the ~1K SWDGE ring)
WRING = 16                   # W2 ring depth
GRING = 6                    # gather-chunk ring depth
NQ = 2                       # SWDGE queues; queue q's desc-gen runs on Q7 cpu pair (2q, 2q+1)


def _wrap16(arr):
    w = arr.reshape(-1, 16).T
    return np.tile(w, (8, 1)).copy()


# Balanced-profile targets: 47 tiles at 16 windows (<=2048 edges), 2 spill
# tiles at 17 (<=2176). Shared across cores so wpt = [16]*47 + [17]*2.
_T_SMALL = 2048
_T_BIG = 2176
_SPILL = (47, 48)


def _balance_dests(percnt):
    """Pack local dests into NTILE tiles (<=128 each) so per-tile edge
    counts fit the shared profile. Returns perm[dest] -> tile*128+rank."""
    target = np.full(NTILE, _T_SMALL, np.int64)
    for t_ in _SPILL:
        target[t_] = _T_BIG
    order = np.argsort(-percnt, kind="stable")
    tile_edges = np.zeros(NTILE, np.int64)
    tile_n = np.zeros(NTILE, np.int64)
    assign = np.full(NLOC, -1, np.int64)
    for dstv in order:
        cands = np.where(tile_n < 128)[0]
        t_ = cands[np.argmax(target[cands] - tile_edges[cands])]
        assign[dstv] = t_
        tile_edges[t_] += percnt[dstv]
        tile_n[t_] += 1
    by_tile = [list(np.where(assign == t_)[0]) for t_ in range(NTILE)]
    for _ in range(4000):
        over = tile_edges - target
        tmax = int(np.argmax(over))
        if over[tmax] <= 0:
            break
        slack = target - tile_edges
        tmin = int(np.argmax(slack))
        cm = percnt[by_tile[tmax]]
        cn = percnt[by_tile[tmin]]
        want = over[tmax] // 2 + 1
        best = None
        bd = 1 << 60
        for i in np.argsort(-cm)[:24]:
            d1 = by_tile[tmax][i]
            j = int(np.argmin(np.abs(cn - (percnt[d1] - want))))
            d2 = by_tile[tmin][j]
            delta = percnt[d1] - percnt[d2]
            if delta <= 0:
                continue
            if abs(delta - want) < bd:
                bd = abs(delta - want)
                best = (d1, d2, delta)
        if best is None:
            break
        d1, d2, delta = best
        by_tile[tmax].remove(d1)
        by_tile[tmin].remove(d2)
        by_tile[tmax].append(d2)
        by_tile[tmin].append(d1)
        assign[d1] = tmin
        assign[d2] = tmax
        tile_edges[tmax] -= delta
        tile_edges[tmin] += delta
    perm = np.full(NLOC, -1, np.int64)
    for t_ in range(NTILE):
        for rank, dstv in enumerate(sorted(by_tile[t_])):
            perm[dstv] = t_ * 128 + rank
    return perm


def preprocess(embed, edge_index, trend):
    row = np.asarray(edge_index[0], dtype=np.int64)
    col = np.asarray(edge_index[1], dtype=np.int64)
    trend = np.asarray(trend, dtype=np.float32)
    core = col // NLOC

    # Per-core dest->tile rebalancing. The agg tables live in PERMUTED,
    # NPAD-padded layout end-to-end: global row of (core c, orig local
    # dest l) is c*NPAD + perm_c[l]. Gather indices below use this
    # layout; the host inverts it in assemble().
    perms = []
    for c in range(NCORES):
        dcl = col[core == c] - c * NLOC
        perms.append(_balance_dests(np.bincount(dcl, minlength=NLOC)))

    grow_of = np.empty(N, dtype=np.int64)  # orig node -> permuted row
    for c in range(NCORES):
        grow_of[c * NLOC:(c + 1) * NLOC] = c * NPAD + perms[c]

    per_core = []
    cnts = np.zeros((NCORES, NTILE), dtype=np.int64)
    for c in range(NCORES):
        m = core == c
        r, dc, t = row[m], col[m] - c * NLOC, trend[m]
        dc = perms[c][dc]                  # permuted local slot
        o = np.argsort(dc, kind="stable")
        r, dc, t = r[o], dc[o], t[o]
        dt = dc // 128
        cnts[c] = np.bincount(dt, minlength=NTILE)
        per_core.append((r, dc, t, dt))

    wpt = np.maximum(1, -(-cnts.max(axis=0) // 128))
    NW = int(wpt.sum())
    NSLOT = NW * 128
    wbase = np.concatenate([[0], np.cumsum(wpt)[:-1]])
    w_dt = np.repeat(np.arange(NTILE), wpt)
    w_first = np.zeros(NW, dtype=bool)
    w_first[wbase] = True
    w_last = np.zeros(NW, dtype=bool)
    w_last[np.cumsum(wpt) - 1] = True

    # Permuted pair table: row grow_of[n] holds embed[n]; pad rows zero.
    trows = NCORES * NPAD
    embed2 = np.zeros((trows, D), dtype=ml_dtypes.bfloat16)
    embed2[grow_of] = np.ascontiguousarray(embed, dtype=np.float32) \
        .astype(ml_dtypes.bfloat16)
    embed2 = embed2.reshape(trows // 2, 2 * D)

    in_maps = []
    for c in range(NCORES):
        r, dc, t, dt = per_core[c]
        starts = np.concatenate([[0], np.cumsum(cnts[c])[:-1]])
        rank = np.arange(len(dc)) - np.repeat(starts, cnts[c])
        slot = wbase[dt] * 128 + rank

        grow = grow_of[r]
        gidx = np.zeros(NSLOT, np.int16)
        dl2 = np.full(NSLOT, 999.0, np.float32)
        tr = np.zeros(NSLOT, np.float32)
        gidx[slot] = (grow // 2).astype(np.int16)
        dl2[slot] = (dc % 128 + 128 * (grow % 2)).astype(np.float32)
        tr[slot] = t

        in_maps.append({
            "embed2": embed2,
            "gidx": _wrap16(gidx),
            "dl2": dl2.reshape(NW, 128).T.copy(),
            "tr": tr.reshape(NW, 128).T.copy(),
            "iota2": np.tile(np.arange(256, dtype=np.float32), (128, 1)),
        })
    sched = dict(NW=NW, NSLOT=NSLOT, w_dt=w_dt, w_first=w_first,
                 w_last=w_last, perms=perms)
    return in_maps, sched


def build(sched):
    NW, NSLOT = sched["NW"], sched["NSLOT"]
    w_dt, w_first, w_last = (sched["w_dt"], sched["w_first"],
                             sched["w_last"])
    NCH = -(-NW // CW)
    ND = -(-NW // 2)   # DVE-produced (even) windows per hop
    NA = NW // 2       # ACT-produced (odd) windows per hop
    chunk_sizes = [min(CW, NW - c * CW) for c in range(NCH)]
    # cumulative windows through global chunk k (k = h*NCH + c)
    cums = []
    tot = 0
    for h in range(HOPS):
        for c in range(NCH):
            tot += chunk_sizes[c]
            cums.append(tot)

    nc = bacc.Bacc("TRN2", target_bir_lowering=False, debug=False,
                   num_devices=NCORES, num_swdge_queues=NQ)

    TROWS2 = NCORES * NPAD // 2   # pair rows of the permuted table
    embed2 = nc.dram_tensor("embed2", [TROWS2, 2 * D], BF16,
                            kind="ExternalInput")
    gidx = nc.dram_tensor("gidx", [128, NSLOT // 16], I16,
                          kind="ExternalInput")
    dl2 = nc.dram_tensor("dl2", [128, NW], F32, kind="ExternalInput")
    tr = nc.dram_tensor("tr", [128, NW], F32, kind="ExternalInput")
    iota2 = nc.dram_tensor("iota2", [128, 256], F32, kind="ExternalInput")
    out3 = nc.dram_tensor("out3", [HOPS, NPAD, D], F32,
                          kind="ExternalOutput")
    aggs = [embed2] + [
        nc.dram_tensor(f"agg{h}", [TROWS2, 2 * D], BF16,
                       addr_space="Shared")
        for h in range(1, HOPS)
    ]
    cc_in = [nc.dram_tensor(f"ccin{h}", [NPAD, D], BF16)
             for h in range(HOPS - 1)]
    rg = [list(range(NCORES))]

    gidx_sb = nc.alloc_sbuf_tensor("gidx_sb", [128, NSLOT // 16], I16)
    dl2_sb = nc.alloc_sbuf_tensor("dl2_sb", [128, NW], F32)
    tr_sb = nc.alloc_sbuf_tensor("tr_sb", [128, NW], F32)
    iota_sb = nc.alloc_sbuf_tensor("iota_sb", [128, 256], F32)
    gt = nc.alloc_sbuf_tensor("gt", [128, GRING, CW, 2 * D], BF16)
    w2 = nc.alloc_sbuf_tensor("w2", [128, WRING, 256], BF16)
    stage = nc.alloc_sbuf_tensor("stage", [128, NTILE, D], F32)
    stage_bf = nc.alloc_sbuf_tensor("stage_bf", [128, NTILE, D], BF16)
    ps = nc.alloc_psum_tensor("ps", [128, NTILE, D], F32)

    s_meta = nc.alloc_semaphore("s_meta")
    s_g = [nc.alloc_semaphore(f"s_g{i}") for i in range(GRING)]
    s_wd = nc.alloc_semaphore("s_wd")
    s_mm = nc.alloc_semaphore("s_mm")
    s_st = nc.alloc_semaphore("s_st")
    s_cv = nc.alloc_semaphore("s_cv")
    s_cc = nc.alloc_semaphore("s_cc")
    s_ag = nc.alloc_semaphore("s_ag")
    s_out = nc.alloc_semaphore("s_out")
    s_pq = [nc.alloc_semaphore(f"s_pq{q}") for q in range(NQ)]
    s_gx = nc.alloc_semaphore("s_gx")

    with nc.Block() as block:

        @block.sync
        def _(sp):
            sp.dma_start(gidx_sb[:], gidx[:]).then_inc(s_gx, 16)
            sp.dma_start(dl2_sb[:], dl2[:]).then_inc(s_meta, 16)
            sp.dma_start(tr_sb[:], tr[:]).then_inc(s_meta, 16)
            sp.dma_start(iota_sb[:], iota2[:]).then_inc(s_meta, 16)
            for h in range(HOPS):
                sp.wait_ge(s_st, h + 1)
                sp.dma_start(
                    out3.ap()[h].rearrange("(t p) d -> p t d", p=128),
                    stage[:]).then_inc(s_out, 16)
                if h < HOPS - 1:
                    sp.wait_ge(s_cv, h + 1)
                    sp.dma_start(
                        cc_in[h].ap().rearrange("(t p) d -> p t d", p=128),
                        stage_bf[:]).then_inc(s_cc, 16)

        @block.gpsimd
        def _(g):
            g.wait_ge(s_gx, 16)

            prep_cnt = [0] * NQ
            trig_cnt = [0] * NQ

            def prep(h, c):
                gc = h * NCH + c
                q = gc % NQ
                if gc >= GRING:
                    g.wait_ge(s_mm, cums[gc - GRING])
                nwc = chunk_sizes[c]
                nidx = nwc * 128
                w0 = c * CW
                g.dma_gather(
                    gt[:, gc % GRING, 0:nwc, :], aggs[h].ap(),
                    gidx_sb[:, w0 * 8:(w0 + nwc) * 8],
                    nidx, nidx, 2 * D, single_packet=False,
                    prepare_only=True, queue_num=q,
                    sem=s_g[gc % GRING]).then_inc(s_pq[q], 1)
                prep_cnt[q] += 1

            def trig(h, c):
                gc = h * NCH + c
                q = gc % NQ
                trig_cnt[q] += 1
                g.wait_ge(s_pq[q], trig_cnt[q])
                g.trigger_dma(count=1, queue_num=q)

            # NPRE-deep prep-ahead: trig(gc) fires a prep issued NPRE
            # iterations earlier, so its EVSEM (which lags the prep's
            # retire by ~15us of desc-write drain) has long fired and the
            # trigger's wait never stalls the sequencer. Next-hop chunks
            # 0..NPRE-1 are prepped before the AllGather (descriptors
            # reference addresses only); their triggers fire after it.
            NPRE = 3
            total = HOPS * NCH

            def hc(gc):
                return gc // NCH, gc % NCH

            for h in range(HOPS):
                if h == 0:
                    for k in range(NPRE):
                        prep(*hc(k))
                for c in range(NCH):
                    gc = h * NCH + c
                    if gc + NPRE < total:
                        prep(*hc(gc + NPRE))
                    trig(h, c)
                if h + 1 < HOPS:
                    g.wait_ge(s_cc, 16 * (h + 1))
                    g.collective_compute(
                        "AllGather", mybir.AluOpType.bypass,
                        replica_groups=rg,
                        ins=[cc_in[h].ap().opt()],
                        outs=[aggs[h + 1].ap().opt()],
                    ).then_inc(s_ag, 1)
                    g.wait_ge(s_ag, h + 1)

        @block.vector
        def _(v):
            v.wait_ge(s_meta, 48)
            for h in range(HOPS):
                for w in range(NW):
                    gw = h * NW + w
                    if gw >= WRING:
                        v.wait_ge(s_mm, gw - WRING + 1)
                    v.tensor_scalar(
                        w2[:, gw % WRING, :], iota_sb[:],
                        dl2_sb[:, w:w + 1], tr_sb[:, w:w + 1],
                        mybir.AluOpType.is_equal,
                        mybir.AluOpType.mult).then_inc(s_wd, 1)
                if h < HOPS - 1:
                    v.wait_ge(s_st, h + 1)
                    if h > 0:
                        v.wait_ge(s_cc, 16 * h)
                    v.tensor_scalar_mul(
                        stage_bf[:], stage[:], 1.0).then_inc(s_cv, 1)

        @block.tensor
        def _(t):
            gw = 0
            for h in range(HOPS):
                if h > 0:
                    t.wait_ge(s_st, h)  # PSUM drained by ACT
                for w in range(NW):
                    c = w // CW
                    gc = h * NCH + c
                    if w % CW == 0:
                        t.wait_ge(s_g[gc % GRING], 16 * (gc // GRING + 1))
                    t.wait_ge(s_wd, h * NW + w + 1)
                    wl = w - c * CW
                    dt_ = int(w_dt[w])
                    t.matmul(
                        ps[:, dt_, :], w2[:, gw % WRING, 0:128],
                        gt[:, gc % GRING, wl, 0:D],
                        start=bool(w_first[w]), stop=False)
                    t.matmul(
                        ps[:, dt_, :], w2[:, gw % WRING, 128:256],
                        gt[:, gc % GRING, wl, D:2 * D],
                        start=False,
                        stop=bool(w_last[w])).then_inc(s_mm, 1)
                    gw += 1

        @block.scalar
        def _(a):
            a.wait_ge(s_meta, 48)
            for h in range(HOPS):
                a.wait_ge(s_mm, NW * (h + 1))
                if h > 0:
                    a.wait_ge(s_out, 16 * h)
                a.copy(stage[:], ps[:]).then_inc(s_st, 1)

    nc.compile()
    return nc


def assemble(embed, results, perms):
    out = np.empty((N, HOPS + 1, D), dtype=np.float32)
    out[:, 0, :] = np.asarray(embed, dtype=np.float32)
    for c in range(NCORES):
        o3 = np.asarray(results[c]["out3"]).reshape(HOPS, NPAD, D)
        sl = slice(c * NLOC, (c + 1) * NLOC)
        for h in range(HOPS):
            out[sl, h + 1, :] = o3[h, perms[c], :]
    return out


def run(embed, edge_index, trend, trace=False, trace_kwargs=None):
    in_maps, sched = preprocess(embed, edge_index, trend)
    nc = build(sched)
    r = run_bass_kernel_spmd(
        nc, in_maps, core_ids=list(range(NCORES)),
        trace=trace, **(trace_kwargs or {}))
    return assemble(embed, r.results, sched["perms"]), r


def kernel(embed, edge_index, trend):
    out, _ = run(embed, edge_index, trend)
    return out

